# revision 12
# baseline (speedup 1.0000x reference)
"""Causal self-attention (B=1, S=4096, D=768, H=12) on 8 TRN2 NeuronCores.

Sharding: 4 head-groups (3 heads each) x 2 query-parity halves; no
collectives. Core c = 2*g + p handles heads [3g, 3g+3) and query rows
{r : r % 2 == p} (strided assignment balances causal work perfectly).

V2 vs baseline:
  - DMA head killed: inputs arrive as few large need-ordered DMAs
    (weights first, x^T in 512-column slices) so the PE starts at ~4us
    instead of ~40us.
  - Causal band truncation: the 8 diagonal blocks per q-tile only
    compute scores/exp/PV on their causally-valid query range; band
    blocks are packed in pairs (1,7),(2,6),(3,5) so each EXP call still
    covers a contiguous 512-col region (ACT has ~300ns/call overhead).
  - The {0,1} mask shrinks to a single [128,64] triangle applied
    in-place to the 64 partial columns of each band block.
  - Independent PE work (next tile's V/Q projections, previous tile's
    out-projection) is interleaved into the EXP-paced attention stream
    so the tensor engine never idles behind the scalar engine.

All matmuls run in bf16 (f32 PSUM accumulation); softmax exp in f32.
"""
import os

import numpy as np
import ml_dtypes

import concourse.bass as bass
import concourse.mybir as mybir
import concourse.tile as tile
from concourse import bacc
from concourse.bass_utils import run_bass_kernel_spmd

BF16 = mybir.dt.bfloat16
F32 = mybir.dt.float32
NPBF16 = ml_dtypes.bfloat16

S = 4096          # sequence length
D = 768           # model dim
HD = 64           # head dim
HL = 3            # heads per core
DL = HL * HD      # 192 local qkv cols per core
SQ = S // 2       # 2048 local queries per core
NQT = 4           # q-tiles per core
QTW = 512         # q-tile width (local queries)
NKB = S // 128    # 32 key blocks of 128
NDC = D // 128    # 6 contraction chunks of 128 over D
VW = HD + 1       # V' column stride per head (64 V cols + ones col)
SCALE = HD ** -0.5

# band packs: diagonal blocks b paired so each pack's widths sum to <=512
BAND_PACKS = ((0,), (1, 7), (2, 6), (3, 5), (4,))


def build_nc():
    nc = bacc.Bacc(None, target_bir_lowering=False)
    xT = nc.declare_dram_parameter("xT", [D, S], BF16, isOutput=False)
    xqT = nc.declare_dram_parameter("xqT", [D, SQ], BF16, isOutput=False)
    wk = nc.declare_dram_parameter("wk", [D, DL], BF16, isOutput=False)
    wq = nc.declare_dram_parameter("wq", [D, DL], BF16, isOutput=False)
    wv = nc.declare_dram_parameter("wv", [D, DL], BF16, isOutput=False)
    bkq = nc.declare_dram_parameter("bkq", [DL, 2], F32, isOutput=False)
    bv = nc.declare_dram_parameter("bv", [DL], F32, isOutput=False)
    wout = nc.declare_dram_parameter("wout", [DL, D], BF16, isOutput=False)
    mask64 = nc.declare_dram_parameter("mask64", [128, 64], BF16, isOutput=False)
    out = nc.declare_dram_parameter("out", [SQ, D], BF16, isOutput=True)

    from contextlib import ExitStack

    with tile.TileContext(nc) as tc, ExitStack() as ctx:
        persist = ctx.enter_context(tc.tile_pool(name="persist", bufs=1))
        xtp = ctx.enter_context(tc.tile_pool(name="xtp", bufs=1))
        wp = ctx.enter_context(tc.tile_pool(name="wp", bufs=1))
        pp = ctx.enter_context(tc.tile_pool(name="pp", bufs=1, space="PSUM"))
        pvp = ctx.enter_context(tc.tile_pool(name="pvp", bufs=1, space="PSUM"))
        psp = ctx.enter_context(tc.tile_pool(name="psp", bufs=2, space="PSUM"))
        pop = ctx.enter_context(tc.tile_pool(name="pop", bufs=1, space="PSUM"))
        ep = ctx.enter_context(tc.tile_pool(name="ep", bufs=3))
        rp = ctx.enter_context(tc.tile_pool(name="rp", bufs=2))
        osb = ctx.enter_context(tc.tile_pool(name="osb", bufs=3))

        kT01 = persist.tile([128, S], BF16)         # K^T heads 0,1
        kT2 = persist.tile([64, S], BF16)           # K^T head 2
        qT01 = persist.tile([128, SQ], BF16)        # Q^T heads 0,1
        qT2 = persist.tile([64, SQ], BF16)
        aT01 = persist.tile([128, SQ], BF16)        # attn^T heads 0,1
        aT2 = persist.tile([64, SQ], BF16)
        vbig = persist.tile([128, NKB * HL * VW], BF16)  # V' blocks [k,195]
        bvb = persist.tile([128, DL], F32)          # bv broadcast over rows
        msk = persist.tile([128, 64], BF16)         # causal triangle r<=2c+p
        ones1 = persist.tile([1, 64], BF16)
        bkq0 = persist.tile([128, 2], F32)
        bkq1 = persist.tile([64, 2], F32)
        wo0 = persist.tile([128, D], BF16)
        wo1 = persist.tile([64, D], BF16)

        nc.vector.memset(vbig, 1.0)
        nc.vector.memset(ones1, 1.0)

        # x^T / xq^T land as 512-column slices holding all 6 contraction
        # chunks: tile cols = kc*512 + j. Weights land as [128, 6*DL].
        xt = [xtp.tile([128, NDC * 512], BF16, name=f"xt{n}") for n in range(8)]
        xq = [xtp.tile([128, NDC * 512], BF16, name=f"xq{t}") for t in range(NQT)]
        wk_t = wp.tile([128, NDC * DL], BF16, name="wk")
        wq_t = wp.tile([128, NDC * DL], BF16, name="wq")
        wv_t = wp.tile([128, NDC * DL], BF16, name="wv")

        xT_r = xT.rearrange("(c p) n -> p c n", p=128)      # [128, 6, 4096]
        xq_r = xqT.rearrange("(c p) n -> p c n", p=128)     # [128, 6, 2048]

        def dma_x(dst, src_r, j0):
            nc.sync.dma_start(
                out=dst.rearrange("p (c n) -> p c n", n=512),
                in_=src_r[:, :, j0:j0 + 512])

        # need-ordered input DMAs (sync queue ~0.7us issue each):
        nc.sync.dma_start(out=wk_t.rearrange("p (c m) -> p c m", m=DL),
                          in_=wk.rearrange("(c p) m -> p c m", p=128))
        dma_x(xt[0], xT_r, 0)
        nc.sync.dma_start(out=wv_t.rearrange("p (c m) -> p c m", m=DL),
                          in_=wv.rearrange("(c p) m -> p c m", p=128))
        dma_x(xt[1], xT_r, 512)
        dma_x(xt[2], xT_r, 1024)
        dma_x(xt[3], xT_r, 1536)
        nc.sync.dma_start(out=wq_t.rearrange("p (c m) -> p c m", m=DL),
                          in_=wq.rearrange("(c p) m -> p c m", p=128))
        dma_x(xq[3], xq_r, 1536)
        dma_x(xt[4], xT_r, 2048)
        dma_x(xt[5], xT_r, 2560)
        dma_x(xt[6], xT_r, 3072)
        dma_x(xt[7], xT_r, 3584)
        dma_x(xq[2], xq_r, 1024)
        dma_x(xq[1], xq_r, 512)
        dma_x(xq[0], xq_r, 0)
        # small tensors on the gpsimd queue (parallel issue path)
        nc.gpsimd.dma_start(out=bkq0, in_=bkq[0:128, :])
        nc.gpsimd.dma_start(out=bkq1, in_=bkq[128:DL, :])
        nc.gpsimd.dma_start(out=bvb, in_=bv[:].partition_broadcast(128))
        nc.gpsimd.dma_start(out=msk, in_=mask64[:, :])
        nc.gpsimd.dma_start(out=wo0, in_=wout[0:128, :])
        nc.gpsimd.dma_start(out=wo1, in_=wout[128:DL, :])

        def kq_proj(dst01, dst2, w_t, rhs, bc, n, m):
            # dst[m-rows, cols n*512..] = W^T x^T + b  for one m-pass
            nsl = slice(n * 512, (n + 1) * 512)
            mw = 128 if m == 0 else 64
            msl = slice(0, 128) if m == 0 else slice(128, DL)
            ps = pp.tile([128, 512], F32, name="pk", tag="pk")
            for kc in range(NDC):
                nc.tensor.matmul(
                    ps[:mw, :],
                    lhsT=w_t[:, kc * DL:(kc + 1) * DL][:, msl],
                    rhs=rhs[:, kc * 512:(kc + 1) * 512],
                    start=(kc == 0), stop=(kc == NDC - 1),
                )
            dst = dst01 if m == 0 else dst2
            bias = (bkq0 if m == 0 else bkq1)[:, bc:bc + 1]
            nc.vector.tensor_scalar_add(
                out=dst[0:mw, nsl], in0=ps[:mw, :], scalar1=bias[:mw, :])

        def v_proj(kb):
            pv = pvp.tile([128, DL], F32, name="pv", tag="pv")
            n, j = kb // 4, (kb % 4) * 128
            for kc in range(NDC):
                nc.tensor.matmul(
                    pv, lhsT=xt[n][:, kc * 512 + j:kc * 512 + j + 128],
                    rhs=wv_t[:, kc * DL:(kc + 1) * DL],
                    start=(kc == 0), stop=(kc == NDC - 1),
                )
            # one strided add writes all 3 heads' V cols (ones col skipped)
            voff = kb * HL * VW
            dstv = vbig[:, voff:voff + HL * VW]
            dstv = dstv.rearrange("p (h vw) -> p h vw", vw=VW)[:, :, 0:HD]
            nc.vector.tensor_add(
                out=dstv,
                in0=pv.rearrange("p (h d) -> p h d", d=HD),
                in1=bvb.rearrange("p (h d) -> p h d", d=HD),
            )

        heads = (
            (kT01[0:64], qT01[0:64], aT01[0:64]),
            (kT01[64:128], qT01[64:128], aT01[64:128]),
            (kT2[0:64], qT2[0:64], aT2[0:64]),
        )

        ot_tiles = {}

        def out_proj_half(qt, ncol, final=False):
            # split at the PSUM-slot reuse boundary so the WAR wait on the
            # previous half's copy never stalls the PE mid-filler
            osl = slice(qt * 128, (qt + 1) * 128)
            if ncol == 0:
                ot_tiles[qt] = osb.tile([128, D], BF16, name="ot", tag="ot")
            ot = ot_tiles[qt]
            cw = 512 if ncol == 0 else 256
            csl = slice(ncol * 512, ncol * 512 + cw)
            if final:  # scores slots are free by now: 2-deep pipelining
                pot = psp.tile([128, 1024], F32, name="ps", tag="ps")
            else:
                pot = psp.tile([128, 512], F32, name="pot", tag="aux1", bufs=1)
            nc.tensor.matmul(
                pot[:, :cw], lhsT=aT01[:, osl], rhs=wo0[:, csl],
                start=True, stop=False, skip_group_check=True)
            nc.tensor.matmul(
                pot[:, :cw], lhsT=aT2[:, osl], rhs=wo1[:, csl],
                start=False, stop=True, skip_group_check=True)
            nc.vector.tensor_copy(out=ot[:, csl], in_=pot[:, :cw])
            if final:
                nc.gpsimd.dma_start(out=out[osl, csl], in_=ot[:, csl])
            elif ncol == 1:
                nc.gpsimd.dma_start(out=out[osl, :], in_=ot)

        def attention(t, fillers):
            def pump(k=1):
                for _ in range(k):
                    if fillers:
                        fillers.pop(0)()

            qoff = t * QTW
            # pack list: (kb, psum_off, width, q_start, is_band)
            packs = []
            for kb0 in range(0, 8 * t, 2):
                packs.append([(kb0, 0, 512, 0, False),
                              (kb0 + 1, 512, 512, 0, False)])
            for pr in BAND_PACKS:
                lst, off = [], 0
                for b in pr:
                    w = 512 - 64 * b
                    lst.append((8 * t + b, off, w, 64 * b, True))
                    off += w
                packs.append(lst)
            last_kb = 8 * t + BAND_PACKS[-1][-1]

            po_of = {}

            def emit_pv(h, pack, eT):
                for (kb, off, w, qs, _band) in pack:
                    voff = kb * HL * VW + h * VW
                    nc.tensor.matmul(
                        po_of[h][0:VW, qs:QTW], lhsT=vbig[:, voff:voff + VW],
                        rhs=eT[:, off:off + w],
                        start=(kb == 0), stop=(kb == last_kb),
                        skip_group_check=True,
                    )

            def divide(h):
                # divide by the softmax sum (row HD of po)
                po = po_of[h]
                sums = rp.tile([1, 512], BF16, name="sums", tag="sums")
                nc.vector.tensor_copy(out=sums, in_=po[HD:VW, :])
                pb = psp.tile([128, 512], F32, name="pb", tag="aux1", bufs=1)
                nc.tensor.matmul(pb[0:64, :], lhsT=ones1, rhs=sums,
                                 start=True, stop=True)
                recb = rp.tile([64, 512], F32, name="recb", tag="recb")
                nc.vector.reciprocal_approx_fast(out=recb, in_=pb[0:64, :])
                nc.vector.tensor_mul(
                    out=heads[h][2][:, qoff:qoff + QTW],
                    in0=po[0:HD, :], in1=recb)

            pend = None  # (h, pack, eT) whose PV is not yet emitted
            for h in range(HL):
                kT_h, qT_h, _aT_h = heads[h]
                po_of[h] = pop.tile([VW, 512], F32, name="po", tag="po")
                for pack in packs:
                    tw = sum(p[2] for p in pack)
                    ps = psp.tile([128, 1024], F32, name="ps", tag="ps")
                    for (kb, off, w, qs, _band) in pack:
                        nc.tensor.matmul(
                            ps[:, off:off + w],
                            lhsT=kT_h[:, kb * 128:(kb + 1) * 128],
                            rhs=qT_h[:, qoff + qs:qoff + QTW],
                            start=True, stop=True,
                        )
                    eT = ep.tile([128, 1024], BF16, name="eT", tag="eT")
                    nc.scalar.activation(
                        out=eT[:, :tw], in_=ps[:, :tw],
                        func=mybir.ActivationFunctionType.Exp, scale=SCALE)
                    for (kb, off, w, qs, band) in pack:
                        if band:  # zero the 64 partial cols of the triangle
                            nc.vector.tensor_mul(
                                out=eT[:, off:off + 64],
                                in0=eT[:, off:off + 64], in1=msk)
                    pump(1)
                    if pend is not None:
                        ph = pend[0]
                        emit_pv(*pend)
                        if ph != h:  # po(ph) now complete
                            divide(ph)
                    pend = (h, pack, eT)
            emit_pv(*pend)
            divide(HL - 1)
            pump(len(fillers))

        # ---- schedule (q-tiles processed t = 3,2,1,0) ----
        # Head: all K and V projections, one dense DMA-fed stream (no PE
        # idle gaps, HAM ramps once), then Q(3). The big PE-bound q-tile
        # runs first; later ACT-bound tiles eat out-proj/Q-proj fillers.
        for n in range(8):
            for m in range(2):
                kq_proj(kT01, kT2, wk_t, xt[n], 0, n, m)
            for kb in range(4 * n, 4 * n + 4):
                v_proj(kb)
        for m in range(2):
            kq_proj(qT01, qT2, wq_t, xq[3], 1, 3, m)

        for t in range(NQT - 1, -1, -1):
            fillers = []
            if t > 0:
                for m in range(2):
                    fillers.append(
                        lambda t=t, m=m: kq_proj(qT01, qT2, wq_t, xq[t - 1],
                                                 1, t - 1, m))
            if t < NQT - 1:
                for qt in range(4 * (t + 1), 4 * (t + 2)):
                    for ncol in range(2):
                        fillers.append(
                            lambda qt=qt, ncol=ncol: out_proj_half(qt, ncol))
            attention(t, fillers)
        # t=0 tile's out-projection: interleave halves so the PSUM-slot
        # WAR (copy of previous half) never blocks the next matmul pair
        for qt, ncol in ((0, 0), (1, 0), (0, 1), (2, 0), (1, 1),
                         (3, 0), (2, 1), (3, 1)):
            out_proj_half(qt, ncol, final=True)

    nc.finalize()
    return nc


_NC_CACHE = {}


def _get_nc():
    if "nc" not in _NC_CACHE:
        _NC_CACHE["nc"] = build_nc()
    return _NC_CACHE["nc"]


def kernel(x, Wqkv, bqkv, Wout, bout):
    x = np.asarray(x, dtype=np.float32)
    Wqkv = np.asarray(Wqkv, dtype=np.float32)
    bqkv = np.asarray(bqkv, dtype=np.float32)
    Wout = np.asarray(Wout, dtype=np.float32)
    bout = np.asarray(bout, dtype=np.float32)
    B, S_, D_ = x.shape
    assert (B, S_, D_) == (1, S, D)
    nc = _get_nc()

    xT_np = np.ascontiguousarray(x[0].T).astype(NPBF16)          # [768, 4096]
    in_maps = []
    for c in range(8):
        g, p = c // 2, c % 2
        csl = slice(DL * g, DL * (g + 1))
        rr = np.arange(128, dtype=np.int64)[:, None]
        cc = np.arange(64, dtype=np.int64)[None, :]
        mask = (rr <= 2 * cc + p).astype(NPBF16)
        bk_h = bqkv[D + DL * g:D + DL * (g + 1)].astype(np.float32)
        bq_h = bqkv[csl].astype(np.float32)
        in_maps.append({
            "xT": xT_np,
            "xqT": np.ascontiguousarray(xT_np[:, p::2]),
            "wk": np.ascontiguousarray(Wqkv[:, D + DL * g:D + DL * (g + 1)]).astype(NPBF16),
            "wq": np.ascontiguousarray(Wqkv[:, csl]).astype(NPBF16),
            "wv": np.ascontiguousarray(Wqkv[:, 2 * D + DL * g:2 * D + DL * (g + 1)]).astype(NPBF16),
            "bkq": np.ascontiguousarray(np.stack([bk_h, bq_h], axis=1)),
            "bv": np.ascontiguousarray(bqkv[2 * D + DL * g:2 * D + DL * (g + 1)]).astype(np.float32),
            "wout": np.ascontiguousarray(Wout[csl, :]).astype(NPBF16),
            "mask64": mask,
        })

    trace = bool(int(os.environ.get("ATTN_TRACE", "0")))
    tmpdir = os.environ.get("ATTN_TMPDIR") or None
    res = run_bass_kernel_spmd(nc, in_maps, core_ids=list(range(8)), trace=trace,
                               tmpdir=tmpdir)
    if trace:
        _NC_CACHE["last_result"] = res

    out_full = np.zeros((S, D), np.float32)
    for p in range(2):
        acc = np.zeros((SQ, D), np.float32)
        for g in range(4):
            acc += res.results[2 * g + p]["out"]
        out_full[p::2] = acc
    out_full += bout.astype(np.float32)[None, :]
    return out_full[None].astype(np.float32)


# revision 14
# speedup vs baseline: 1.2469x; 1.2469x over previous
"""Causal self-attention (B=1, S=4096, D=768, H=12) on 8 TRN2 NeuronCores.

Sharding: 4 head-groups (3 heads each) x 2 query-parity halves; no
collectives. Core c = 2*g + p handles heads [3g, 3g+3) and query rows
{r : r % 2 == p} (strided assignment balances causal work perfectly).

V2 vs baseline:
  - DMA head killed: inputs arrive as few large need-ordered DMAs
    (weights first, x^T in 512-column slices) so the PE starts at ~4us
    instead of ~40us.
  - Causal band truncation: the 8 diagonal blocks per q-tile only
    compute scores/exp/PV on their causally-valid query range; band
    blocks are packed in pairs (1,7),(2,6),(3,5) so each EXP call still
    covers a contiguous 512-col region (ACT has ~300ns/call overhead).
  - The {0,1} mask shrinks to a single [128,64] triangle applied
    in-place to the 64 partial columns of each band block.
  - Independent PE work (next tile's V/Q projections, previous tile's
    out-projection) is interleaved into the EXP-paced attention stream
    so the tensor engine never idles behind the scalar engine.

All matmuls run in bf16 (f32 PSUM accumulation); softmax exp in f32.
"""
import os

import numpy as np
import ml_dtypes

import concourse.bass as bass
import concourse.mybir as mybir
import concourse.tile as tile
from concourse import bacc
from concourse.bass_utils import run_bass_kernel_spmd

BF16 = mybir.dt.bfloat16
F32 = mybir.dt.float32
NPBF16 = ml_dtypes.bfloat16

S = 4096          # sequence length
D = 768           # model dim
HD = 64           # head dim
HL = 3            # heads per core
DL = HL * HD      # 192 local qkv cols per core
SQ = S // 2       # 2048 local queries per core
NQT = 4           # q-tiles per core
QTW = 512         # q-tile width (local queries)
NKB = S // 128    # 32 key blocks of 128
NDC = D // 128    # 6 contraction chunks of 128 over D
VW = HD + 1       # V' column stride per head (64 V cols + ones col)
SCALE = HD ** -0.5

# band packs: diagonal blocks b paired so each pack's widths sum to <=512
BAND_PACKS = ((0,), (1, 7), (2, 6), (3, 5), (4,))


def build_nc():
    nc = bacc.Bacc(None, target_bir_lowering=False)
    xT = nc.declare_dram_parameter("xT", [D, S], BF16, isOutput=False)
    xqT = nc.declare_dram_parameter("xqT", [D, SQ], BF16, isOutput=False)
    wk = nc.declare_dram_parameter("wk", [D, DL], BF16, isOutput=False)
    wq = nc.declare_dram_parameter("wq", [D, DL], BF16, isOutput=False)
    wv = nc.declare_dram_parameter("wv", [D, DL], BF16, isOutput=False)
    bkq = nc.declare_dram_parameter("bkq", [DL, 2], F32, isOutput=False)
    bv = nc.declare_dram_parameter("bv", [DL], F32, isOutput=False)
    wout = nc.declare_dram_parameter("wout", [DL, D], BF16, isOutput=False)
    mask64 = nc.declare_dram_parameter("mask64", [128, 64], BF16, isOutput=False)
    out = nc.declare_dram_parameter("out", [SQ, D], BF16, isOutput=True)

    from contextlib import ExitStack

    with tile.TileContext(nc) as tc, ExitStack() as ctx:
        persist = ctx.enter_context(tc.tile_pool(name="persist", bufs=1))
        xtp = ctx.enter_context(tc.tile_pool(name="xtp", bufs=1))
        wp = ctx.enter_context(tc.tile_pool(name="wp", bufs=1))
        pp = ctx.enter_context(tc.tile_pool(name="pp", bufs=1, space="PSUM"))
        pvp = ctx.enter_context(tc.tile_pool(name="pvp", bufs=1, space="PSUM"))
        psp = ctx.enter_context(tc.tile_pool(name="psp", bufs=2, space="PSUM"))
        pop = ctx.enter_context(tc.tile_pool(name="pop", bufs=1, space="PSUM"))
        ep = ctx.enter_context(tc.tile_pool(name="ep", bufs=3))
        rp = ctx.enter_context(tc.tile_pool(name="rp", bufs=2))
        osb = ctx.enter_context(tc.tile_pool(name="osb", bufs=3))

        kT01 = persist.tile([128, S], BF16)         # K^T heads 0,1
        kT2 = persist.tile([64, S], BF16)           # K^T head 2
        qT01 = persist.tile([128, SQ], BF16)        # Q^T heads 0,1
        qT2 = persist.tile([64, SQ], BF16)
        aT01 = persist.tile([128, SQ], BF16)        # attn^T heads 0,1
        aT2 = persist.tile([64, SQ], BF16)
        vbig = persist.tile([128, NKB * HL * VW], BF16)  # V' blocks [k,195]
        bvb = persist.tile([128, DL], F32)          # bv broadcast over rows
        msk = persist.tile([128, 64], BF16)         # causal triangle r<=2c+p
        ones1 = persist.tile([1, 64], BF16)
        bkq0 = persist.tile([128, 2], F32)
        bkq1 = persist.tile([64, 2], F32)
        wo0 = persist.tile([128, D], BF16)
        wo1 = persist.tile([64, D], BF16)

        nc.vector.memset(vbig, 1.0)
        nc.vector.memset(ones1, 1.0)

        # x^T / xq^T land as 512-column slices holding all 6 contraction
        # chunks: tile cols = kc*512 + j. Weights land as [128, 6*DL].
        xt = [xtp.tile([128, NDC * 512], BF16, name=f"xt{n}") for n in range(8)]
        xq = [xtp.tile([128, NDC * 512], BF16, name=f"xq{t}") for t in range(NQT)]
        wk_t = wp.tile([128, NDC * DL], BF16, name="wk")
        wq_t = wp.tile([128, NDC * DL], BF16, name="wq")
        wv_t = wp.tile([128, NDC * DL], BF16, name="wv")

        xT_r = xT.rearrange("(c p) n -> p c n", p=128)      # [128, 6, 4096]
        xq_r = xqT.rearrange("(c p) n -> p c n", p=128)     # [128, 6, 2048]

        def dma_x(dst, src_r, j0):
            nc.sync.dma_start(
                out=dst.rearrange("p (c n) -> p c n", n=512),
                in_=src_r[:, :, j0:j0 + 512])

        # need-ordered input DMAs (sync queue ~0.7us issue each):
        nc.sync.dma_start(out=wk_t.rearrange("p (c m) -> p c m", m=DL),
                          in_=wk.rearrange("(c p) m -> p c m", p=128))
        dma_x(xt[0], xT_r, 0)
        nc.sync.dma_start(out=wv_t.rearrange("p (c m) -> p c m", m=DL),
                          in_=wv.rearrange("(c p) m -> p c m", p=128))
        dma_x(xt[1], xT_r, 512)
        nc.sync.dma_start(out=wq_t.rearrange("p (c m) -> p c m", m=DL),
                          in_=wq.rearrange("(c p) m -> p c m", p=128))
        dma_x(xq[0], xq_r, 0)
        dma_x(xt[2], xT_r, 1024)
        dma_x(xt[3], xT_r, 1536)
        dma_x(xq[1], xq_r, 512)
        dma_x(xt[4], xT_r, 2048)
        dma_x(xt[5], xT_r, 2560)
        dma_x(xq[2], xq_r, 1024)
        dma_x(xt[6], xT_r, 3072)
        dma_x(xt[7], xT_r, 3584)
        dma_x(xq[3], xq_r, 1536)
        # small tensors on the gpsimd queue (parallel issue path)
        nc.gpsimd.dma_start(out=bkq0, in_=bkq[0:128, :])
        nc.gpsimd.dma_start(out=bkq1, in_=bkq[128:DL, :])
        nc.gpsimd.dma_start(out=bvb, in_=bv[:].partition_broadcast(128))
        nc.gpsimd.dma_start(out=msk, in_=mask64[:, :])
        nc.gpsimd.dma_start(out=wo0, in_=wout[0:128, :])
        nc.gpsimd.dma_start(out=wo1, in_=wout[128:DL, :])

        def kq_proj(dst01, dst2, w_t, rhs, bc, n, m):
            # dst[m-rows, cols n*512..] = W^T x^T + b  for one m-pass
            nsl = slice(n * 512, (n + 1) * 512)
            mw = 128 if m == 0 else 64
            msl = slice(0, 128) if m == 0 else slice(128, DL)
            ps = pp.tile([128, 512], F32, name="pk", tag="pk")
            for kc in range(NDC):
                nc.tensor.matmul(
                    ps[:mw, :],
                    lhsT=w_t[:, kc * DL:(kc + 1) * DL][:, msl],
                    rhs=rhs[:, kc * 512:(kc + 1) * 512],
                    start=(kc == 0), stop=(kc == NDC - 1),
                )
            dst = dst01 if m == 0 else dst2
            bias = (bkq0 if m == 0 else bkq1)[:, bc:bc + 1]
            nc.vector.tensor_scalar_add(
                out=dst[0:mw, nsl], in0=ps[:mw, :], scalar1=bias[:mw, :])

        def v_proj(kb):
            pv = pvp.tile([128, DL], F32, name="pv", tag="pv")
            n, j = kb // 4, (kb % 4) * 128
            for kc in range(NDC):
                nc.tensor.matmul(
                    pv, lhsT=xt[n][:, kc * 512 + j:kc * 512 + j + 128],
                    rhs=wv_t[:, kc * DL:(kc + 1) * DL],
                    start=(kc == 0), stop=(kc == NDC - 1),
                )
            # one strided add writes all 3 heads' V cols (ones col skipped)
            voff = kb * HL * VW
            dstv = vbig[:, voff:voff + HL * VW]
            dstv = dstv.rearrange("p (h vw) -> p h vw", vw=VW)[:, :, 0:HD]
            nc.vector.tensor_add(
                out=dstv,
                in0=pv.rearrange("p (h d) -> p h d", d=HD),
                in1=bvb.rearrange("p (h d) -> p h d", d=HD),
            )

        heads = (
            (kT01[0:64], qT01[0:64], aT01[0:64]),
            (kT01[64:128], qT01[64:128], aT01[64:128]),
            (kT2[0:64], qT2[0:64], aT2[0:64]),
        )

        ot_tiles = {}

        def out_proj_half(qt, ncol, final=False):
            # split at the PSUM-slot reuse boundary so the WAR wait on the
            # previous half's copy never stalls the PE mid-filler
            osl = slice(qt * 128, (qt + 1) * 128)
            if ncol == 0:
                ot_tiles[qt] = osb.tile([128, D], BF16, name="ot", tag="ot")
            ot = ot_tiles[qt]
            cw = 512 if ncol == 0 else 256
            csl = slice(ncol * 512, ncol * 512 + cw)
            if final:  # scores slots are free by now: 2-deep pipelining
                pot = psp.tile([128, 1024], F32, name="ps", tag="ps")
            else:
                pot = psp.tile([128, 512], F32, name="pot", tag="aux1", bufs=1)
            nc.tensor.matmul(
                pot[:, :cw], lhsT=aT01[:, osl], rhs=wo0[:, csl],
                start=True, stop=False, skip_group_check=True)
            nc.tensor.matmul(
                pot[:, :cw], lhsT=aT2[:, osl], rhs=wo1[:, csl],
                start=False, stop=True, skip_group_check=True)
            nc.vector.tensor_copy(out=ot[:, csl], in_=pot[:, :cw])
            if final:
                nc.gpsimd.dma_start(out=out[osl, csl], in_=ot[:, csl])
            elif ncol == 1:
                nc.gpsimd.dma_start(out=out[osl, :], in_=ot)

        def attention(t, fillers):
            def pump(k=1):
                for _ in range(k):
                    if fillers:
                        fillers.pop(0)()

            qoff = t * QTW
            # pack list: (kb, psum_off, width, q_start, is_band)
            packs = []
            for kb0 in range(0, 8 * t, 2):
                packs.append([(kb0, 0, 512, 0, False),
                              (kb0 + 1, 512, 512, 0, False)])
            for pr in BAND_PACKS:
                lst, off = [], 0
                for b in pr:
                    w = 512 - 64 * b
                    lst.append((8 * t + b, off, w, 64 * b, True))
                    off += w
                packs.append(lst)
            last_kb = 8 * t + BAND_PACKS[-1][-1]

            po_of = {}

            def emit_pv(h, pack, eT):
                for (kb, off, w, qs, _band) in pack:
                    voff = kb * HL * VW + h * VW
                    nc.tensor.matmul(
                        po_of[h][0:VW, qs:QTW], lhsT=vbig[:, voff:voff + VW],
                        rhs=eT[:, off:off + w],
                        start=(kb == 0), stop=(kb == last_kb),
                        skip_group_check=True,
                    )

            def divide(h):
                # divide by the softmax sum (row HD of po)
                po = po_of[h]
                sums = rp.tile([1, 512], BF16, name="sums", tag="sums")
                nc.vector.tensor_copy(out=sums, in_=po[HD:VW, :])
                pb = psp.tile([128, 512], F32, name="pb", tag="aux1", bufs=1)
                nc.tensor.matmul(pb[0:64, :], lhsT=ones1, rhs=sums,
                                 start=True, stop=True)
                recb = rp.tile([64, 512], F32, name="recb", tag="recb")
                nc.vector.reciprocal_approx_fast(out=recb, in_=pb[0:64, :])
                nc.vector.tensor_mul(
                    out=heads[h][2][:, qoff:qoff + QTW],
                    in0=po[0:HD, :], in1=recb)

            pend = None  # (h, pack, eT) whose PV is not yet emitted
            for h in range(HL):
                kT_h, qT_h, _aT_h = heads[h]
                po_of[h] = pop.tile([VW, 512], F32, name="po", tag="po")
                for pack in packs:
                    tw = sum(p[2] for p in pack)
                    ps = psp.tile([128, 1024], F32, name="ps", tag="ps")
                    for (kb, off, w, qs, _band) in pack:
                        nc.tensor.matmul(
                            ps[:, off:off + w],
                            lhsT=kT_h[:, kb * 128:(kb + 1) * 128],
                            rhs=qT_h[:, qoff + qs:qoff + QTW],
                            start=True, stop=True,
                        )
                    eT = ep.tile([128, 1024], BF16, name="eT", tag="eT")
                    nc.scalar.activation(
                        out=eT[:, :tw], in_=ps[:, :tw],
                        func=mybir.ActivationFunctionType.Exp, scale=SCALE)
                    for (kb, off, w, qs, band) in pack:
                        if band:  # zero the 64 partial cols of the triangle
                            nc.vector.tensor_mul(
                                out=eT[:, off:off + 64],
                                in0=eT[:, off:off + 64], in1=msk)
                    pump(1)
                    if pend is not None:
                        ph = pend[0]
                        emit_pv(*pend)
                        if ph != h:  # po(ph) now complete
                            divide(ph)
                    pend = (h, pack, eT)
            emit_pv(*pend)
            divide(HL - 1)
            pump(len(fillers))

        # ---- schedule: minimal head, then q-tiles t=0..3 with fillers ----
        # Head: just enough for attention(0) to start (~12us). Everything
        # else becomes filler inside the EXP-paced attention phases, sized
        # to each phase's ACT-PE deficit so the PE never idles (keeps the
        # HAM clock at 2.4GHz).
        def K(n, m):
            return lambda: kq_proj(kT01, kT2, wk_t, xt[n], 0, n, m)

        def Q(t, m):
            return lambda: kq_proj(qT01, qT2, wq_t, xq[t], 1, t, m)

        def V(kb):
            return lambda: v_proj(kb)

        def O(qt, ncol):
            return lambda: out_proj_half(qt, ncol)

        for f in [K(0, 0), K(0, 1), K(1, 0), K(1, 1)]:
            f()
        for kb in range(8):
            v_proj(kb)
        Q(0, 0)()
        Q(0, 1)()

        FILL = {
            0: [K(2, 0), K(2, 1), V(8), V(9), K(3, 0), K(3, 1),
                V(10), V(11), V(12), V(13), V(14), V(15), Q(1, 0), Q(1, 1)],
            1: [K(4, 0), K(4, 1), V(16), V(17), K(5, 0), K(5, 1),
                V(18), V(19), Q(2, 0), Q(2, 1)],
            2: [K(6, 0), K(6, 1), V(20), V(21), K(7, 0), K(7, 1),
                V(22), V(23), Q(3, 0), Q(3, 1)],
            3: [V(24), V(25), V(26), V(27), V(28), V(29), V(30), V(31),
                O(0, 0), O(0, 1), O(1, 0), O(1, 1), O(4, 0), O(4, 1),
                O(2, 0), O(2, 1), O(3, 0), O(3, 1), O(5, 0), O(5, 1),
                O(6, 0), O(6, 1), O(7, 0), O(7, 1), O(8, 0), O(8, 1),
                O(9, 0), O(9, 1), O(10, 0), O(10, 1), O(11, 0), O(11, 1)],
        }
        for t in range(NQT):
            attention(t, FILL[t])
        # last tile's out-projection: interleave halves so the PSUM-slot
        # WAR (copy of previous half) never blocks the next matmul pair
        for qt, ncol in ((12, 0), (13, 0), (12, 1), (14, 0), (13, 1),
                         (15, 0), (14, 1), (15, 1)):
            out_proj_half(qt, ncol, final=True)

    nc.finalize()
    return nc


_NC_CACHE = {}


def _get_nc():
    if "nc" not in _NC_CACHE:
        _NC_CACHE["nc"] = build_nc()
    return _NC_CACHE["nc"]


def kernel(x, Wqkv, bqkv, Wout, bout):
    x = np.asarray(x, dtype=np.float32)
    Wqkv = np.asarray(Wqkv, dtype=np.float32)
    bqkv = np.asarray(bqkv, dtype=np.float32)
    Wout = np.asarray(Wout, dtype=np.float32)
    bout = np.asarray(bout, dtype=np.float32)
    B, S_, D_ = x.shape
    assert (B, S_, D_) == (1, S, D)
    nc = _get_nc()

    xT_np = np.ascontiguousarray(x[0].T).astype(NPBF16)          # [768, 4096]
    in_maps = []
    for c in range(8):
        g, p = c // 2, c % 2
        csl = slice(DL * g, DL * (g + 1))
        rr = np.arange(128, dtype=np.int64)[:, None]
        cc = np.arange(64, dtype=np.int64)[None, :]
        mask = (rr <= 2 * cc + p).astype(NPBF16)
        bk_h = bqkv[D + DL * g:D + DL * (g + 1)].astype(np.float32)
        bq_h = bqkv[csl].astype(np.float32)
        in_maps.append({
            "xT": xT_np,
            "xqT": np.ascontiguousarray(xT_np[:, p::2]),
            "wk": np.ascontiguousarray(Wqkv[:, D + DL * g:D + DL * (g + 1)]).astype(NPBF16),
            "wq": np.ascontiguousarray(Wqkv[:, csl]).astype(NPBF16),
            "wv": np.ascontiguousarray(Wqkv[:, 2 * D + DL * g:2 * D + DL * (g + 1)]).astype(NPBF16),
            "bkq": np.ascontiguousarray(np.stack([bk_h, bq_h], axis=1)),
            "bv": np.ascontiguousarray(bqkv[2 * D + DL * g:2 * D + DL * (g + 1)]).astype(np.float32),
            "wout": np.ascontiguousarray(Wout[csl, :]).astype(NPBF16),
            "mask64": mask,
        })

    trace = bool(int(os.environ.get("ATTN_TRACE", "0")))
    tmpdir = os.environ.get("ATTN_TMPDIR") or None
    res = run_bass_kernel_spmd(nc, in_maps, core_ids=list(range(8)), trace=trace,
                               tmpdir=tmpdir)
    if trace:
        _NC_CACHE["last_result"] = res

    out_full = np.zeros((S, D), np.float32)
    for p in range(2):
        acc = np.zeros((SQ, D), np.float32)
        for g in range(4):
            acc += res.results[2 * g + p]["out"]
        out_full[p::2] = acc
    out_full += bout.astype(np.float32)[None, :]
    return out_full[None].astype(np.float32)
